# revision 7
# baseline (speedup 1.0000x reference)
"""Additive attention kernel for Trainium2, data-parallel over batch on 8 cores.

Reference computation (per batch b):
    q_proj = query @ W2 + b2                      # [U]
    v_proj = values[b] @ W1 + b1                  # [T, U]
    score  = tanh(v_proj + q_proj) @ V + bv       # [T, 1]
    attw   = softmax(score, axis=0)               # [T, 1]
    ctx    = sum(attw * values[b], axis=0)        # [D]

Device strategy (per core, BL=4 batches):
  - values tiles are cast fp32->bf16 during the HBM DMA (SWDGE cast), then
    transposed on-chip via the DMA xbar (dma_start_transpose) so the
    contraction dim D lands on partitions.
  - v_projT = W1.T-chunks (stationary) x valuesT (moving) accumulated in PSUM,
    tanh+bias applied by the scalar engine (bias = q_projT column + b1 + b2,
    per-partition), score matvec with V on the tensor engine (M=1 matmuls).
  - softmax skips the max-subtraction: |score| <= sum|V| (tanh bounded), so
    exp cannot overflow; softmax is shift-invariant so results match.
  - context = expw.T x values via tensor engine, with expw moved from a row
    to columns using K=1 matmuls against a [[1.0]] constant.

bv is ignored: softmax(score + bv) == softmax(score).
"""

import numpy as np

B, T, D, U = 32, 2048, 1024, 1024
NCORES = 8
BL = B // NCORES  # batches per core
KC = D // 128     # contraction chunks
MC = U // 128     # u chunks
NH = 2            # halves per batch (score pass granularity)
TH = T // NH      # rows per half
NN = TH // 512    # 512-col chunks per half
TT = T // 128     # 128-row tiles per batch

_CACHE = {}


def _build_module():
    from contextlib import ExitStack

    import concourse.bass as bass
    import concourse.tile as tile
    from concourse import bacc, mybir
    from concourse.bass import ts

    from concourse.masks import make_identity

    f32 = mybir.dt.float32
    bf16 = mybir.dt.bfloat16
    Tanh = mybir.ActivationFunctionType.Tanh
    Exp = mybir.ActivationFunctionType.Exp
    X = mybir.AxisListType.X

    nc = bacc.Bacc(
        "TRN2", target_bir_lowering=False, debug=False, num_devices=NCORES
    )
    values = nc.dram_tensor("values", [BL, T, D], f32, kind="ExternalInput").ap()
    query = nc.dram_tensor("query", [BL, D], f32, kind="ExternalInput").ap()
    W1d = nc.dram_tensor("W1", [D, U], f32, kind="ExternalInput").ap()
    W2d = nc.dram_tensor("W2", [D, U], f32, kind="ExternalInput").ap()
    b1d = nc.dram_tensor("b1", [U], f32, kind="ExternalInput").ap()
    b2d = nc.dram_tensor("b2", [U], f32, kind="ExternalInput").ap()
    Vd = nc.dram_tensor("V", [U, 1], f32, kind="ExternalInput").ap()
    ctx_out = nc.dram_tensor("ctx", [BL, D], f32, kind="ExternalOutput").ap()
    attw_out = nc.dram_tensor("attw", [BL, T], f32, kind="ExternalOutput").ap()

    with tile.TileContext(nc) as tc, ExitStack() as ctx:
        consts = ctx.enter_context(tc.tile_pool(name="consts", bufs=1))
        psum_misc = ctx.enter_context(
            tc.tile_pool(name="psm", bufs=2, space="PSUM")
        )

        one_bf = consts.tile([1, 1], bf16)
        nc.vector.memset(one_bf, 1.0)
        eye_bl = consts.tile([BL, BL], f32)
        make_identity(nc, eye_bl)
        eye_mc = consts.tile([MC, MC], f32)
        make_identity(nc, eye_mc)

        # ---- phase 0: weights, biases, q_proj ----
        w1 = consts.tile([128, KC, U], bf16)
        vb = consts.tile([128, MC], bf16)
        qb = consts.tile([128, MC, BL], f32)  # tanh bias: q_projT + b1 + b2

        with tc.tile_pool(name="stage", bufs=2) as stage:
            # W1 -> bf16 [p, k, u] with d = k*128 + p
            for h in range(2):
                st = stage.tile([128, KC // 2, U], f32, tag="wstage")
                nc.sync.dma_start(
                    out=st,
                    in_=W1d.rearrange("(k p) u -> p k u", p=128)[
                        :, h * (KC // 2) : (h + 1) * (KC // 2), :
                    ],
                )
                for k in range(KC // 2):
                    nc.vector.tensor_copy(
                        out=w1[:, h * (KC // 2) + k, :], in_=st[:, k, :]
                    )
            # W2 -> bf16 (scoped; only needed for q_proj)
            w2 = stage.tile([128, KC, U], bf16, tag="w2")
            for h in range(2):
                st = stage.tile([128, KC // 2, U], f32, tag="wstage")
                nc.sync.dma_start(
                    out=st,
                    in_=W2d.rearrange("(k p) u -> p k u", p=128)[
                        :, h * (KC // 2) : (h + 1) * (KC // 2), :
                    ],
                )
                for k in range(KC // 2):
                    nc.vector.tensor_copy(
                        out=w2[:, h * (KC // 2) + k, :], in_=st[:, k, :]
                    )
            # V -> bf16 [p, m] with u = m*128 + p (strided but tiny)
            vst = stage.tile([128, MC], f32, tag="vst")
            nc.sync.dma_start(
                out=vst, in_=Vd.rearrange("(m p) o -> p (m o)", p=128)
            )
            nc.vector.tensor_copy(out=vb, in_=vst)

            # b1/b2 natural [MC, 128]; transpose via K=1 matmuls, summed in PSUM
            b1n = stage.tile([MC, 128], f32, tag="b1n")
            nc.sync.dma_start(out=b1n, in_=b1d.rearrange("(m p) -> m p", p=128))
            b2n = stage.tile([MC, 128], f32, tag="b2n")
            nc.sync.dma_start(out=b2n, in_=b2d.rearrange("(m p) -> m p", p=128))
            b12_ps = psum_misc.tile([128, MC], f32, tag="mm")
            nc.tensor.matmul(
                b12_ps, lhsT=b1n, rhs=eye_mc, start=True, stop=False
            )
            nc.tensor.matmul(
                b12_ps, lhsT=b2n, rhs=eye_mc, start=False, stop=True
            )
            b12 = stage.tile([128, MC], f32, tag="b12")
            nc.vector.tensor_copy(out=b12, in_=b12_ps)

            # queryT via K=1 matmuls: qt[p, k, b] = query[b, k*128+p]
            qn = stage.tile([BL, D], f32, tag="qn")
            nc.sync.dma_start(out=qn, in_=query)
            qt_ps = psum_misc.tile([128, KC, BL], f32, tag="mm")
            for k in range(KC):
                nc.tensor.matmul(
                    qt_ps[:, k, :],
                    lhsT=qn[:, ts(k, 128)],
                    rhs=eye_bl,
                    start=True,
                    stop=True,
                )
            qt = stage.tile([128, KC, BL], bf16, tag="qt")
            nc.vector.tensor_copy(out=qt, in_=qt_ps)

            # q_projT chunks + bias fold
            for m in range(MC):
                qp_ps = psum_misc.tile([128, BL], f32, tag="mm")
                for k in range(KC):
                    nc.tensor.matmul(
                        qp_ps,
                        lhsT=w2[:, k, ts(m, 128)],
                        rhs=qt[:, k, :],
                        start=(k == 0),
                        stop=(k == KC - 1),
                    )
                nc.vector.tensor_scalar_add(
                    out=qb[:, m, :], in0=qp_ps, scalar1=b12[:, m : m + 1]
                )

        # ---- main pools ----
        vnat_pool = ctx.enter_context(tc.tile_pool(name="vnat", bufs=2 * TT))
        vt_pool = ctx.enter_context(tc.tile_pool(name="vt", bufs=2))
        tanh_pool = ctx.enter_context(tc.tile_pool(name="tanh", bufs=4))
        sm_pool = ctx.enter_context(tc.tile_pool(name="sm", bufs=2))
        psum_v = ctx.enter_context(tc.tile_pool(name="psv", bufs=2, space="PSUM"))
        psum_s = ctx.enter_context(tc.tile_pool(name="pss", bufs=1, space="PSUM"))
        psum_c = ctx.enter_context(tc.tile_pool(name="psc", bufs=1, space="PSUM"))

        for b in range(BL):
            # load + cast values[b] tiles (SWDGE casts fp32->bf16 in flight)
            vnat = []
            for i in range(TT):
                vt_i = vnat_pool.tile([128, D], bf16, tag="vnat")
                nc.gpsimd.dma_start(out=vt_i, in_=values[b, ts(i, 128), :])
                vnat.append(vt_i)

            ew_f = sm_pool.tile([1, T], f32, tag="ewf")

            for half in range(NH):
                # transpose half's tiles: vt[p, tt, j, t] = v[t, j*128+p]
                vt = vt_pool.tile([128, TH // 128, KC, 128], bf16, tag="vt")
                for tt in range(TH // 128):
                    nc.sync.dma_start(
                        out=vt[:, tt, :, :],
                        in_=vnat[half * (TH // 128) + tt][:],
                        transpose=True,
                    )
                sc_ps = psum_s.tile([1, NN, 512], f32, tag="sc")
                for m in range(MC):
                    for n in range(NN):
                        pv = psum_v.tile([128, 512], f32, tag="pv")
                        for k in range(KC):
                            nc.tensor.matmul(
                                pv,
                                lhsT=w1[:, k, ts(m, 128)],
                                rhs=vt[:, 4 * n : 4 * n + 4, k, :],
                                start=(k == 0),
                                stop=(k == KC - 1),
                            )
                        th = tanh_pool.tile([128, 512], bf16, tag="th")
                        nc.scalar.activation(
                            out=th,
                            in_=pv,
                            func=Tanh,
                            bias=qb[:, m, b : b + 1],
                            scale=1.0,
                        )
                        nc.tensor.matmul(
                            sc_ps[0:1, n, :],
                            lhsT=vb[:, m : m + 1],
                            rhs=th,
                            start=(m == 0),
                            stop=(m == MC - 1),
                        )
                for n in range(NN):
                    off = half * TH + n * 512
                    nc.scalar.activation(
                        out=ew_f[0:1, off : off + 512],
                        in_=sc_ps[0:1, n, :],
                        func=Exp,
                    )

            # softmax normalize (no max-subtraction needed; scores bounded)
            l_sb = sm_pool.tile([1, 1], f32, tag="l")
            nc.vector.reduce_sum(out=l_sb, in_=ew_f, axis=X)
            rl = sm_pool.tile([1, 1], f32, tag="rl")
            nc.vector.reciprocal(out=rl, in_=l_sb)
            nc.vector.tensor_scalar_mul(out=ew_f, in0=ew_f, scalar1=rl)
            nc.sync.dma_start(out=attw_out[b, :], in_=ew_f)

            ew_bf = sm_pool.tile([1, T], bf16, tag="ewbf")
            nc.vector.tensor_copy(out=ew_bf, in_=ew_f)

            # move weight row to columns: ewT[p, j] = w[j*128 + p]
            ewT_ps = psum_misc.tile([128, TT], f32, tag="mm")
            for j in range(TT):
                nc.tensor.matmul(
                    ewT_ps[:, j : j + 1],
                    lhsT=ew_bf[0:1, ts(j, 128)],
                    rhs=one_bf,
                    start=True,
                    stop=True,
                )
            ew_t = sm_pool.tile([128, TT], bf16, tag="ewt")
            nc.vector.tensor_copy(out=ew_t, in_=ewT_ps)

            # context = sum_t w_t * values[t, :]
            ctx_ps = psum_c.tile([1, 2, 512], f32, tag="cx")
            for j in range(TT):
                for h2 in range(2):
                    nc.tensor.matmul(
                        ctx_ps[0:1, h2, :],
                        lhsT=ew_t[:, j : j + 1],
                        rhs=vnat[j][:, ts(h2, 512)],
                        start=(j == 0),
                        stop=(j == TT - 1),
                    )
            ctx_sb = sm_pool.tile([1, D], f32, tag="ctxsb")
            for h2 in range(2):
                nc.vector.tensor_copy(
                    out=ctx_sb[0:1, ts(h2, 512)], in_=ctx_ps[0:1, h2, :]
                )
            nc.sync.dma_start(out=ctx_out[b, :], in_=ctx_sb)

    nc.compile()
    return nc


def _get_module():
    if "nc" not in _CACHE:
        _CACHE["nc"] = _build_module()
    return _CACHE["nc"]


def kernel(query, values, W1, b1, W2, b2, V, bv):
    from concourse import bass_utils

    nc = _get_module()

    query = np.ascontiguousarray(np.asarray(query, dtype=np.float32))
    values = np.ascontiguousarray(np.asarray(values, dtype=np.float32))
    W1 = np.ascontiguousarray(np.asarray(W1, dtype=np.float32))
    W2 = np.ascontiguousarray(np.asarray(W2, dtype=np.float32))
    b1 = np.ascontiguousarray(np.asarray(b1, dtype=np.float32))
    b2 = np.ascontiguousarray(np.asarray(b2, dtype=np.float32))
    V = np.ascontiguousarray(np.asarray(V, dtype=np.float32))

    in_maps = []
    for c in range(NCORES):
        sl = slice(c * BL, (c + 1) * BL)
        in_maps.append(
            {
                "values": values[sl],
                "query": query[sl],
                "W1": W1,
                "W2": W2,
                "b1": b1,
                "b2": b2,
                "V": V,
            }
        )

    import os

    trace = bool(int(os.environ.get("KERNEL_TRACE", "0")))
    kw = {}
    if os.environ.get("KERNEL_TMPDIR"):
        kw["tmpdir"] = os.environ["KERNEL_TMPDIR"]
    res = bass_utils.run_bass_kernel_spmd(
        nc, in_maps, core_ids=list(range(NCORES)), trace=trace, **kw
    )
    _CACHE["last_res"] = res
    ctx = np.concatenate([res.results[c]["ctx"] for c in range(NCORES)], axis=0)
    attw = np.concatenate(
        [res.results[c]["attw"] for c in range(NCORES)], axis=0
    )
    return ctx.astype(np.float32), attw.reshape(B, T, 1).astype(np.float32)
